# revision 48
# baseline (speedup 1.0000x reference)
"""ClusterMemory loss kernel for 8 TRN2 NeuronCores.

Problem: loss = label-smoothed CE over logits = [prototype/T, (x_norm @ features.T)/T]
  B=256, D=2048, N=65536, P=4096, T=0.05, EPS=0.1.

Sharding strategy (per the row-wise memory-bank hint):
  - features [N, D] row-sharded: core c owns rows [c*8192, (c+1)*8192).
    The shard is passed host-transposed as featT [D, 8192] so the device
    reads it with the contraction dim on partitions (contiguous 2KB bursts).
  - prototype column-sharded: core c owns cols [c*512, (c+1)*512).
  - inputs/targets replicated (targets as per-core local one-hot compare keys).

Per-core device program:
  1. normalize x rows, fold 1/TEMP into the scale, transpose to xT via PE.
  2. stream featT in 16 slices of [2048, 512]; for each slice and each batch
     half: 16 fp32 matmuls accumulate mem_logits tile [128b, 512n] in PSUM.
     From PSUM directly: row-sum (DVE), exp-sum vs per-core max (ACT fused
     accum), and the target logit via iota==target one-hot (DVE fused
     tensor_tensor_reduce). mem logits never round-trip through SBUF.
  3. prototype slice: max/sum/exp-sum with scale 20.
  4. AllGather 8x [4,2,128] per-core stats (max, sumexp, sum, tgt) -> each
     core merges (online-softmax merge) and computes the scalar loss.
"""

import os
import sys

for _p in ("/opt/trn_rl_repo",):
    if _p not in sys.path:
        sys.path.append(_p)

import numpy as np

B, D, N, P = 256, 2048, 65536, 4096
TEMP = 0.05
EPS = 0.1
NCORES = 8
NSH = N // NCORES          # 8192 memory rows per core
PSH = P // NCORES          # 512 prototype cols per core
NSLICES = 16               # feature slices per core
SN = NSH // NSLICES        # 512 columns per slice
NH = 2                     # batch halves of 128
KC = D // 128              # 16 contraction chunks

_COMPILED = None
LAST_RESULTS = None
# Debug bisect: 0=prep only, 1=+proto, 2=+main loop, 3=full (default)
_STAGE = int(os.environ.get("KSTAGE", "3"))
_DBG_NSLICES = int(os.environ.get("KNSLICES", str(NSLICES)))
_DBG_OPS = os.environ.get("KOPS", "sum,exp,tgt").split(",")
_MERGE = os.environ.get("KMERGE", "host")


def _build():
    import concourse.bacc as bacc
    import concourse.tile as tile
    import concourse.mybir as mybir
    import concourse.masks as masks

    f32 = mybir.dt.float32
    f32r = mybir.dt.float32r
    AF = mybir.ActivationFunctionType
    ALU = mybir.AluOpType
    AX = mybir.AxisListType

    nc = bacc.Bacc("TRN2", target_bir_lowering=False, debug=False,
                   num_devices=NCORES)
    f32r = mybir.dt.float32r

    x_ext = nc.declare_dram_parameter("x", [B, D], f32, isOutput=False)
    # featT host-retiled to [slice, partition, kchunk, n]: per (slice, p) the
    # (kc, f) run is 32KB contiguous in DRAM AND in the SBUF partition row,
    # so each slice DMA is 128 descriptors x 32KB (line-rate).
    ft_ext = nc.declare_dram_parameter("featT", [NSLICES, 128, KC, SN], f32r,
                                       isOutput=False)
    pr_ext = nc.declare_dram_parameter("proto", [B, PSH], f32, isOutput=False)
    ta_ext = nc.declare_dram_parameter("tgtadj", [B, NSLICES], f32, isOutput=False)
    io_ext = nc.declare_dram_parameter("iota", [128, SN], f32, isOutput=False)
    out_shape = [128, 4 * NH] if _MERGE == "host" else [1, 1]
    out_ext = nc.declare_dram_parameter("out", out_shape, f32, isOutput=True)

    # p-major so the pack DMA is contiguous per partition (32B runs)
    cc_in = nc.dram_tensor("cc_in", [128, 4 * NH], f32)
    cc_out = nc.dram_tensor("cc_out", [NCORES, 128, 4 * NH], f32,
                            addr_space="Shared")

    def emit(tc, constp, xp, ftp, statp, xnp, scr, mkp, smallp,
             psp, pspt, psp2):
        ident = constp.tile([128, 128], f32)
        masks.make_identity(nc, ident[:])
        ones = constp.tile([128, 1], f32)
        nc.gpsimd.memset(ones[:], 1.0)
        zero_sc = constp.tile([128, 1], f32)
        nc.gpsimd.memset(zero_sc[:], 0.0)
        iota_sb = constp.tile([128, SN], f32)
        nc.sync.dma_start(iota_sb[:], io_ext[:])
        ta_sb = constp.tile([128, NH, NSLICES], f32)
        nc.sync.dma_start(ta_sb[:], ta_ext[:].rearrange("(h p) t -> p h t", p=128))

        def finish(src):
            out_sb = smallp.tile([1, 1], f32, tag="outsb")
            nc.scalar.activation(out_sb[:], src, AF.Copy)
            nc.sync.dma_start(out_ext[:], out_sb[:])

        # ---- x: load, norms, scale by rnorm/TEMP, transpose to xT ----
        x_sb = xp.tile([128, NH, D], f32)
        nc.sync.dma_start(x_sb[:], x_ext[:].rearrange("(h p) d -> p h d", p=128))
        pr_sb = xp.tile([128, NH, PSH], f32)
        nc.sync.dma_start(pr_sb[:], pr_ext[:].rearrange("(h p) n -> p h n", p=128))

        xT = xp.tile([128, KC * NH * 128], f32r)   # [d-part, (kc,h,b128)]
        negM = []   # per half: -(max(20*pmax, 20)) for exp biasing
        Mst = []    # per half: the max stat itself
        sums = []   # per half: [128, 17] raw logit sums (col 16 = proto)
        esums = []  # per half: [128, 17] exp sums      (col 16 = proto)
        tvals = []  # per half: [128, 16] target-logit partials

        for h in range(NH):
            xh = x_sb[:, h, :]
            xn = xnp.tile([128, D], f32, tag="xn")
            ss = smallp.tile([128, 1], f32, tag=f"ss{h}")
            nc.scalar.activation(xn[:], xh, AF.Square, accum_out=ss[:])
            rs = smallp.tile([128, 1], f32, tag=f"rs{h}")
            nc.vector.reciprocal(rs[:], ss[:])
            rn = smallp.tile([128, 1], f32, tag=f"rn{h}")
            nc.scalar.activation(rn[:], rs[:], AF.Sqrt)  # 1/||x||
            rnt = smallp.tile([128, 1], f32, tag=f"rnt{h}")
            nc.vector.tensor_scalar_mul(rnt[:], rn[:], 1.0 / TEMP)
            nc.vector.tensor_scalar(xn[:], xh, rnt[:], None, ALU.mult)
            # transpose 16 chunks of [128,128] -> xT
            for g in range(KC // 4):
                pst = pspt.tile([128, 512], f32, tag="tps")
                for j in range(4):
                    kc = g * 4 + j
                    nc.tensor.transpose(
                        pst[:, j * 128:(j + 1) * 128],
                        xn[:, kc * 128:(kc + 1) * 128], ident[:])
                for j in range(4):
                    kc = g * 4 + j
                    q = (kc * NH + h) * 128
                    nc.vector.tensor_copy(xT[:, q:q + 128],
                                          pst[:, j * 128:(j + 1) * 128])

            # ---- prototype slice stats ----
            ph = pr_sb[:, h, :]
            pmax = smallp.tile([128, 1], f32, tag=f"pmax{h}")
            nc.vector.tensor_reduce(pmax[:], ph, AX.X, ALU.max)
            praw = smallp.tile([128, 1], f32, tag=f"praw{h}")
            nc.vector.tensor_reduce(praw[:], ph, AX.X, ALU.add)
            M_h = smallp.tile([128, 1], f32, tag=f"M{h}")
            nc.vector.tensor_scalar(M_h[:], pmax[:], 1.0 / TEMP, 1.0 / TEMP,
                                    ALU.mult, ALU.max)
            nM_h = smallp.tile([128, 1], f32, tag=f"nM{h}")
            nc.vector.tensor_scalar_mul(nM_h[:], M_h[:], -1.0)
            negM.append(nM_h)
            Mst.append(M_h)

            sums_h = statp.tile([128, NSLICES + 1], f32, tag=f"sums{h}")
            esums_h = statp.tile([128, NSLICES + 1], f32, tag=f"esums{h}")
            tvals_h = statp.tile([128, NSLICES], f32, tag=f"tvals{h}")
            sums.append(sums_h)
            esums.append(esums_h)
            tvals.append(tvals_h)

            nc.vector.tensor_scalar_mul(sums_h[:, NSLICES:NSLICES + 1],
                                        praw[:], 1.0 / TEMP)
            pej = scr.tile([128, PSH], f32, tag="pej")
            nc.scalar.activation(pej[:], ph, AF.Exp, bias=nM_h[:],
                                 scale=1.0 / TEMP,
                                 accum_out=esums_h[:, NSLICES:NSLICES + 1])

        if _STAGE == 0:
            finish(xT[:1, :1])
            return
        if _STAGE == 1:
            finish(esums[0][:1, :1])
            return

        # ---- main loop: stream featT slices ----
        for s in range(_DBG_NSLICES):
            ft = ftp.tile([128, KC, SN], f32r, tag="ft")
            # two half-DMAs: matmuls for kc<8 start after the first half, so
            # PE idle gaps stay under the ~3.4us HAM re-throttle window
            nc.sync.dma_start(ft[:, :KC // 2, :], ft_ext[s, :, :KC // 2, :])
            nc.sync.dma_start(ft[:, KC // 2:, :], ft_ext[s, :, KC // 2:, :])
            for h in range(NH):
                ps = psp.tile([128, SN], f32, tag="mm")
                for kc in range(KC):
                    q = (kc * NH + h) * 128
                    # float32r: same f32 bytes, single-pass PE mode at
                    # 1 cyc/row (exact fp32 mode is 4 cyc/row).
                    nc.tensor.matmul(ps[:], xT[:, q:q + 128], ft[:, kc, :],
                                     start=(kc == 0), stop=(kc == KC - 1))
                # raw sum
                if "sum" in _DBG_OPS:
                    nc.vector.tensor_reduce(sums[h][:, s:s + 1], ps[:],
                                            AX.X, ALU.add)
                # exp-sum (vs per-core max M)
                if "exp" in _DBG_OPS:
                    ej = scr.tile([128, SN], f32, tag="ej")
                    nc.scalar.activation(ej[:], ps[:], AF.Exp, bias=negM[h][:],
                                         accum_out=esums[h][:, s:s + 1])
                # target pick: one-hot(iota == tgtadj) . mem
                # NOTE: PSUM operand must be in0 of tensor_tensor (in1=PSUM
                # faults the DVE); tensor_tensor_reduce faults outright.
                if "tgt" in _DBG_OPS:
                    mk = mkp.tile([128, SN], f32, tag="mk")
                    nc.vector.tensor_scalar(mk[:], iota_sb[:],
                                            ta_sb[:, h, s:s + 1], None,
                                            ALU.is_equal)
                    tj = scr.tile([128, SN], f32, tag="tj")
                    nc.vector.tensor_tensor(tj[:], ps[:], mk[:], ALU.mult)
                    nc.vector.tensor_reduce(tvals[h][:, s:s + 1], tj[:],
                                            AX.X, ALU.add)
                if not _DBG_OPS or _DBG_OPS == [""]:
                    nc.vector.tensor_reduce(sums[h][:, s:s + 1], ps[:],
                                            AX.X, ALU.add)

        if _STAGE == 2:
            finish(esums[0][:1, :1])
            return

        # ---- local stat totals, pack for AllGather ----
        stats_sb = smallp.tile([128, 4, NH], f32)
        for h in range(NH):
            nc.vector.tensor_copy(stats_sb[:, 0, h:h + 1], Mst[h][:])
            nc.vector.tensor_reduce(stats_sb[:, 1, h:h + 1], esums[h][:],
                                    AX.X, ALU.add)
            nc.vector.tensor_reduce(stats_sb[:, 2, h:h + 1], sums[h][:],
                                    AX.X, ALU.add)
            nc.vector.tensor_reduce(stats_sb[:, 3, h:h + 1], tvals[h][:],
                                    AX.X, ALU.add)
        if _MERGE == "host":
            nc.sync.dma_start(out_ext[:],
                              stats_sb[:].rearrange("p st h -> p (st h)"))
            return
        nc.sync.dma_start(cc_in[:], stats_sb[:].rearrange("p st h -> p (st h)"))

        nc.gpsimd.collective_compute(
            "AllGather", ALU.bypass,
            replica_groups=[list(range(NCORES))],
            ins=[cc_in[:].opt()],
            outs=[cc_out[:].opt()],
        )

        # cc_out rows are core-major [c][p][stat]; transpose strided [8, 128]
        # blocks via PE to get [128b, 8c] tiles per (stat, half).
        raw8 = smallp.tile([NCORES, 128, 4 * NH], f32)
        nc.sync.dma_start(raw8[:].rearrange("c p q -> c (p q)"),
                          cc_out[:].rearrange("c p q -> c (p q)"))

        # ---- merge + loss ----
        # (both halves' Exp before both Ln: avoid ACT table-set swaps)
        fin_ps = psp2.tile([1, 1], f32)
        mrg, mgs, sadjs, exp_insts = [], [], [], []
        for h in range(NH):
            merged = smallp.tile([128, 4, NCORES], f32, tag=f"merged{h}")
            for st in range(4):
                pst8 = pspt.tile([128, 512], f32, tag="tps")
                nc.tensor.transpose(
                    pst8[:, :NCORES],
                    raw8[:, :, st * NH + h],
                    ident[:NCORES, :NCORES])
                nc.vector.tensor_copy(merged[:, st, :], pst8[:, :NCORES])
            mrg.append(merged)
            mg = smallp.tile([128, 1], f32, tag=f"mg{h}")
            nc.vector.tensor_reduce(mg[:], merged[:, 0, :], AX.X, ALU.max)
            mgs.append(mg)
            nmg = smallp.tile([128, 1], f32, tag=f"nmg{h}")
            nc.vector.tensor_scalar_mul(nmg[:], mg[:], -1.0)
            adj = smallp.tile([128, NCORES], f32, tag=f"adj{h}")
            exp_insts.append(
                nc.scalar.activation(adj[:], merged[:, 0, :], AF.Exp, bias=nmg[:]))
            j8 = smallp.tile([128, NCORES], f32, tag=f"j8{h}")
            nc.vector.tensor_tensor(j8[:], adj[:], merged[:, 1, :], ALU.mult)
            sadj = smallp.tile([128, 1], f32, tag=f"sadj{h}")
            nc.vector.tensor_reduce(sadj[:], j8[:], AX.X, ALU.add)
            sadjs.append(sadj)
        for h in range(NH):
            merged, mg, sadj = mrg[h], mgs[h], sadjs[h]
            lg = smallp.tile([128, 1], f32, tag=f"lg{h}")
            lg_inst = nc.scalar.activation(lg[:], sadj[:], AF.Ln)
            # keep both Exp ops before any Ln: one ACT table-set swap, not 3
            tile.add_dep_helper(lg_inst.ins, exp_insts[-1].ins, sync=False,
                                reason="group ACT table sets")
            lse = smallp.tile([128, 1], f32, tag=f"lse{h}")
            nc.vector.tensor_tensor(lse[:], lg[:], mg[:], ALU.add)
            tg = smallp.tile([128, 1], f32, tag=f"tg{h}")
            nc.vector.tensor_reduce(tg[:], merged[:, 3, :], AX.X, ALU.add)
            sg = smallp.tile([128, 1], f32, tag=f"sg{h}")
            nc.vector.tensor_reduce(sg[:], merged[:, 2, :], AX.X, ALU.add)
            a1 = smallp.tile([128, 1], f32, tag=f"a1{h}")
            nc.vector.tensor_scalar(a1[:], tg[:], -(1.0 - EPS), None, ALU.mult)
            a2 = smallp.tile([128, 1], f32, tag=f"a2{h}")
            nc.vector.tensor_scalar(a2[:], sg[:], -EPS / (P + N), None, ALU.mult)
            a3 = smallp.tile([128, 1], f32, tag=f"a3{h}")
            nc.vector.tensor_tensor(a3[:], lse[:], a1[:], ALU.add)
            lossv = smallp.tile([128, 1], f32, tag=f"loss{h}")
            nc.vector.tensor_tensor(lossv[:], a3[:], a2[:], ALU.add)
            nc.tensor.matmul(fin_ps[:], lossv[:], ones[:],
                             start=(h == 0), stop=(h == NH - 1))

        out_sb = smallp.tile([1, 1], f32, tag="outsb")
        nc.scalar.activation(out_sb[:], fin_ps[:], AF.Copy, scale=1.0 / B)
        nc.sync.dma_start(out_ext[:], out_sb[:])

    with tile.TileContext(nc) as tc:
        with (
            tc.tile_pool(name="const", bufs=1) as constp,
            tc.tile_pool(name="xp", bufs=1) as xp,
            tc.tile_pool(name="ft", bufs=3) as ftp,
            tc.tile_pool(name="stats", bufs=1) as statp,
            tc.tile_pool(name="xnp", bufs=2) as xnp,
            tc.tile_pool(name="junk", bufs=2) as scr,
            tc.tile_pool(name="mkp", bufs=2) as mkp,
            tc.tile_pool(name="small", bufs=1) as smallp,
            tc.tile_pool(name="psum", bufs=4, space="PSUM") as psp,
            tc.tile_pool(name="psumt", bufs=2, space="PSUM") as pspt,
            tc.tile_pool(name="psum2", bufs=1, space="PSUM") as psp2,
        ):
            emit(tc, constp, xp, ftp, statp, xnp, scr, mkp, smallp,
                 psp, pspt, psp2)

    nc.compile()
    return nc


def _get_compiled():
    global _COMPILED
    if _COMPILED is None:
        _COMPILED = _build()
    return _COMPILED


def kernel(inputs, targets, prototype, features):
    global LAST_RESULTS
    from concourse.bass_utils import run_bass_kernel_spmd

    inputs = np.ascontiguousarray(np.asarray(inputs, dtype=np.float32))
    prototype = np.ascontiguousarray(np.asarray(prototype, dtype=np.float32))
    features = np.asarray(features, dtype=np.float32)
    tgt = np.asarray(targets).astype(np.int64)

    iota = np.broadcast_to(np.arange(SN, dtype=np.float32), (128, SN)).copy()

    in_maps = []
    for c in range(NCORES):
        # [s, p, kc, f] tiling of features[shard].T (see kernel builder)
        featT = np.ascontiguousarray(
            features[c * NSH:(c + 1) * NSH, :].T
            .reshape(KC, 128, NSLICES, SN).transpose(2, 1, 0, 3))
        # tgtadj[b, t] = local column index target would have in slice t
        tl = tgt - c * NSH
        tgtadj = (tl[:, None] - SN * np.arange(NSLICES)[None, :]).astype(np.float32)
        in_maps.append({
            "x": inputs,
            "featT": featT,
            "proto": np.ascontiguousarray(prototype[:, c * PSH:(c + 1) * PSH]),
            "tgtadj": np.ascontiguousarray(tgtadj),
            "iota": iota,
        })

    nc = _get_compiled()
    res = run_bass_kernel_spmd(
        nc, in_maps, core_ids=list(range(NCORES)),
        trace=bool(os.environ.get("BASS_TRACE")),
    )
    LAST_RESULTS = res
    if _MERGE == "host":
        # gather per-core softmax stats [128, (st,h)] and merge
        st = np.stack([res.results[c]["out"] for c in range(NCORES)])  # [8,128,8]
        st = st.reshape(NCORES, 128, 4, NH).transpose(0, 2, 3, 1)      # [c,st,h,p]
        m, s, sm, t = (st[:, i].reshape(NCORES, B) for i in range(4))  # [c, b]
        mg = m.max(0)
        lse = mg + np.log((s * np.exp(m - mg)).sum(0))
        loss = (lse - (1 - EPS) * t.sum(0) - (EPS / (P + N)) * sm.sum(0)).mean()
        return np.float32(loss)
    return np.float32(res.results[0]["out"].reshape(()))


# revision 49
# speedup vs baseline: 1.1479x; 1.1479x over previous
"""ClusterMemory loss kernel for 8 TRN2 NeuronCores.

Problem: loss = label-smoothed CE over logits = [prototype/T, (x_norm @ features.T)/T]
  B=256, D=2048, N=65536, P=4096, T=0.05, EPS=0.1.

Sharding strategy (per the row-wise memory-bank hint):
  - features [N, D] row-sharded: core c owns rows [c*8192, (c+1)*8192).
    The shard is passed host-transposed as featT [D, 8192] so the device
    reads it with the contraction dim on partitions (contiguous 2KB bursts).
  - prototype column-sharded: core c owns cols [c*512, (c+1)*512).
  - inputs/targets replicated (targets as per-core local one-hot compare keys).

Per-core device program:
  1. normalize x rows, fold 1/TEMP into the scale, transpose to xT via PE.
  2. stream featT in 16 slices of [2048, 512]; for each slice and each batch
     half: 16 fp32 matmuls accumulate mem_logits tile [128b, 512n] in PSUM.
     From PSUM directly: row-sum (DVE), exp-sum vs per-core max (ACT fused
     accum), and the target logit via iota==target one-hot (DVE fused
     tensor_tensor_reduce). mem logits never round-trip through SBUF.
  3. prototype slice: max/sum/exp-sum with scale 20.
  4. AllGather 8x [4,2,128] per-core stats (max, sumexp, sum, tgt) -> each
     core merges (online-softmax merge) and computes the scalar loss.
"""

import os
import sys

for _p in ("/opt/trn_rl_repo",):
    if _p not in sys.path:
        sys.path.append(_p)

import numpy as np

B, D, N, P = 256, 2048, 65536, 4096
TEMP = 0.05
EPS = 0.1
NCORES = 8
NSH = N // NCORES          # 8192 memory rows per core
PSH = P // NCORES          # 512 prototype cols per core
NSLICES = 16               # feature slices per core
SN = NSH // NSLICES        # 512 columns per slice
NH = 2                     # batch halves of 128
KC = D // 128              # 16 contraction chunks

_COMPILED = None
LAST_RESULTS = None
# Debug bisect: 0=prep only, 1=+proto, 2=+main loop, 3=full (default)
_STAGE = int(os.environ.get("KSTAGE", "3"))
_DBG_NSLICES = int(os.environ.get("KNSLICES", str(NSLICES)))
_DBG_OPS = os.environ.get("KOPS", "sum,exp,tgt").split(",")
_MERGE = os.environ.get("KMERGE", "host")


def _build():
    import concourse.bacc as bacc
    import concourse.tile as tile
    import concourse.mybir as mybir
    import concourse.masks as masks

    f32 = mybir.dt.float32
    f32r = mybir.dt.float32r
    AF = mybir.ActivationFunctionType
    ALU = mybir.AluOpType
    AX = mybir.AxisListType

    nc = bacc.Bacc("TRN2", target_bir_lowering=False, debug=False,
                   num_devices=NCORES)
    f32r = mybir.dt.float32r

    x_ext = nc.declare_dram_parameter("x", [B, D], f32, isOutput=False)
    # featT host-retiled to [slice, partition, kchunk, n]: per (slice, p) the
    # (kc, f) run is 32KB contiguous in DRAM AND in the SBUF partition row,
    # so each slice DMA is 128 descriptors x 32KB (line-rate).
    ft_ext = nc.declare_dram_parameter("featT", [NSLICES, 128, KC, SN], f32r,
                                       isOutput=False)
    pr_ext = nc.declare_dram_parameter("proto", [B, PSH], f32, isOutput=False)
    ta_ext = nc.declare_dram_parameter("tgtadj", [B, NSLICES], f32, isOutput=False)
    io_ext = nc.declare_dram_parameter("iota", [128, SN], f32, isOutput=False)
    out_shape = [128, 4 * NH] if _MERGE == "host" else [1, 1]
    out_ext = nc.declare_dram_parameter("out", out_shape, f32, isOutput=True)

    # p-major so the pack DMA is contiguous per partition (32B runs)
    cc_in = nc.dram_tensor("cc_in", [128, 4 * NH], f32)
    cc_out = nc.dram_tensor("cc_out", [NCORES, 128, 4 * NH], f32,
                            addr_space="Shared")

    def emit(tc, constp, xp, ftp, statp, xnp, scr, mkp, smallp,
             psp, pspt, psp2):
        ident = constp.tile([128, 128], f32)
        masks.make_identity(nc, ident[:])
        ones = constp.tile([128, 1], f32)
        nc.gpsimd.memset(ones[:], 1.0)
        zero_sc = constp.tile([128, 1], f32)
        nc.gpsimd.memset(zero_sc[:], 0.0)
        iota_sb = constp.tile([128, SN], f32)
        nc.sync.dma_start(iota_sb[:], io_ext[:])
        ta_sb = constp.tile([128, NH, NSLICES], f32)
        nc.sync.dma_start(ta_sb[:], ta_ext[:].rearrange("(h p) t -> p h t", p=128))

        def finish(src):
            out_sb = smallp.tile([1, 1], f32, tag="outsb")
            nc.scalar.activation(out_sb[:], src, AF.Copy)
            nc.sync.dma_start(out_ext[:], out_sb[:])

        # ---- x: load, norms, scale by rnorm/TEMP, transpose to xT ----
        x_sb = xp.tile([128, NH, D], f32)
        nc.sync.dma_start(x_sb[:], x_ext[:].rearrange("(h p) d -> p h d", p=128))
        pr_sb = xp.tile([128, NH, PSH], f32)
        nc.sync.dma_start(pr_sb[:], pr_ext[:].rearrange("(h p) n -> p h n", p=128))

        xT = xp.tile([128, KC * NH * 128], f32r)   # [d-part, (kc,h,b128)]
        negM = []   # per half: -(max(20*pmax, 20)) for exp biasing
        Mst = []    # per half: the max stat itself
        sums = []   # per half: [128, 17] raw logit sums (col 16 = proto)
        esums = []  # per half: [128, 17] exp sums      (col 16 = proto)
        tvals = []  # per half: [128, 16] target-logit partials

        for h in range(NH):
            xh = x_sb[:, h, :]
            xn = xnp.tile([128, D], f32, tag="xn")
            ss = smallp.tile([128, 1], f32, tag=f"ss{h}")
            nc.scalar.activation(xn[:], xh, AF.Square, accum_out=ss[:])
            rs = smallp.tile([128, 1], f32, tag=f"rs{h}")
            nc.vector.reciprocal(rs[:], ss[:])
            rn = smallp.tile([128, 1], f32, tag=f"rn{h}")
            nc.scalar.activation(rn[:], rs[:], AF.Sqrt)  # 1/||x||
            rnt = smallp.tile([128, 1], f32, tag=f"rnt{h}")
            nc.vector.tensor_scalar_mul(rnt[:], rn[:], 1.0 / TEMP)
            nc.vector.tensor_scalar(xn[:], xh, rnt[:], None, ALU.mult)
            # transpose 16 chunks of [128,128] -> xT
            for g in range(KC // 4):
                pst = pspt.tile([128, 512], f32, tag="tps")
                for j in range(4):
                    kc = g * 4 + j
                    nc.tensor.transpose(
                        pst[:, j * 128:(j + 1) * 128],
                        xn[:, kc * 128:(kc + 1) * 128], ident[:])
                for j in range(4):
                    kc = g * 4 + j
                    q = (kc * NH + h) * 128
                    nc.vector.tensor_copy(xT[:, q:q + 128],
                                          pst[:, j * 128:(j + 1) * 128])

            # ---- prototype slice stats ----
            ph = pr_sb[:, h, :]
            pmax = smallp.tile([128, 1], f32, tag=f"pmax{h}")
            nc.vector.tensor_reduce(pmax[:], ph, AX.X, ALU.max)
            praw = smallp.tile([128, 1], f32, tag=f"praw{h}")
            nc.vector.tensor_reduce(praw[:], ph, AX.X, ALU.add)
            M_h = smallp.tile([128, 1], f32, tag=f"M{h}")
            nc.vector.tensor_scalar(M_h[:], pmax[:], 1.0 / TEMP, 1.0 / TEMP,
                                    ALU.mult, ALU.max)
            nM_h = smallp.tile([128, 1], f32, tag=f"nM{h}")
            nc.vector.tensor_scalar_mul(nM_h[:], M_h[:], -1.0)
            negM.append(nM_h)
            Mst.append(M_h)

            sums_h = statp.tile([128, NSLICES + 1], f32, tag=f"sums{h}")
            esums_h = statp.tile([128, NSLICES + 1], f32, tag=f"esums{h}")
            tvals_h = statp.tile([128, NSLICES], f32, tag=f"tvals{h}")
            sums.append(sums_h)
            esums.append(esums_h)
            tvals.append(tvals_h)

            nc.vector.tensor_scalar_mul(sums_h[:, NSLICES:NSLICES + 1],
                                        praw[:], 1.0 / TEMP)
            pej = scr.tile([128, PSH], f32, tag="pej")
            nc.scalar.activation(pej[:], ph, AF.Exp, bias=nM_h[:],
                                 scale=1.0 / TEMP,
                                 accum_out=esums_h[:, NSLICES:NSLICES + 1])

        if _STAGE == 0:
            finish(xT[:1, :1])
            return
        if _STAGE == 1:
            finish(esums[0][:1, :1])
            return

        # ---- main loop: stream featT slices ----
        for s in range(_DBG_NSLICES):
            ft = ftp.tile([128, KC, SN], f32r, tag="ft")
            # quarter-DMAs: matmuls start after the first quarter, so PE
            # idle gaps stay under the ~3.4us HAM re-throttle window
            for qq in range(4):
                nc.sync.dma_start(ft[:, qq * KC // 4:(qq + 1) * KC // 4, :],
                                  ft_ext[s, :, qq * KC // 4:(qq + 1) * KC // 4, :])
            for h in range(NH):
                ps = psp.tile([128, SN], f32, tag="mm")
                for kc in range(KC):
                    q = (kc * NH + h) * 128
                    # float32r: same f32 bytes, single-pass PE mode at
                    # 1 cyc/row (exact fp32 mode is 4 cyc/row).
                    nc.tensor.matmul(ps[:], xT[:, q:q + 128], ft[:, kc, :],
                                     start=(kc == 0), stop=(kc == KC - 1))
                # raw sum
                if "sum" in _DBG_OPS:
                    nc.vector.tensor_reduce(sums[h][:, s:s + 1], ps[:],
                                            AX.X, ALU.add)
                # exp-sum (vs per-core max M)
                if "exp" in _DBG_OPS:
                    ej = scr.tile([128, SN], f32, tag="ej")
                    nc.scalar.activation(ej[:], ps[:], AF.Exp, bias=negM[h][:],
                                         accum_out=esums[h][:, s:s + 1])
                # target pick: one-hot(iota == tgtadj) . mem
                # NOTE: PSUM operand must be in0 of tensor_tensor (in1=PSUM
                # faults the DVE); tensor_tensor_reduce faults outright.
                if "tgt" in _DBG_OPS:
                    mk = mkp.tile([128, SN], f32, tag="mk")
                    nc.vector.tensor_scalar(mk[:], iota_sb[:],
                                            ta_sb[:, h, s:s + 1], None,
                                            ALU.is_equal)
                    tj = scr.tile([128, SN], f32, tag="tj")
                    nc.vector.tensor_tensor(tj[:], ps[:], mk[:], ALU.mult)
                    nc.vector.tensor_reduce(tvals[h][:, s:s + 1], tj[:],
                                            AX.X, ALU.add)
                if not _DBG_OPS or _DBG_OPS == [""]:
                    nc.vector.tensor_reduce(sums[h][:, s:s + 1], ps[:],
                                            AX.X, ALU.add)

        if _STAGE == 2:
            finish(esums[0][:1, :1])
            return

        # ---- local stat totals, pack for AllGather ----
        stats_sb = smallp.tile([128, 4, NH], f32)
        for h in range(NH):
            nc.vector.tensor_copy(stats_sb[:, 0, h:h + 1], Mst[h][:])
            nc.vector.tensor_reduce(stats_sb[:, 1, h:h + 1], esums[h][:],
                                    AX.X, ALU.add)
            nc.vector.tensor_reduce(stats_sb[:, 2, h:h + 1], sums[h][:],
                                    AX.X, ALU.add)
            nc.vector.tensor_reduce(stats_sb[:, 3, h:h + 1], tvals[h][:],
                                    AX.X, ALU.add)
        if _MERGE == "host":
            nc.sync.dma_start(out_ext[:],
                              stats_sb[:].rearrange("p st h -> p (st h)"))
            return
        nc.sync.dma_start(cc_in[:], stats_sb[:].rearrange("p st h -> p (st h)"))

        nc.gpsimd.collective_compute(
            "AllGather", ALU.bypass,
            replica_groups=[list(range(NCORES))],
            ins=[cc_in[:].opt()],
            outs=[cc_out[:].opt()],
        )

        # cc_out rows are core-major [c][p][stat]; transpose strided [8, 128]
        # blocks via PE to get [128b, 8c] tiles per (stat, half).
        raw8 = smallp.tile([NCORES, 128, 4 * NH], f32)
        nc.sync.dma_start(raw8[:].rearrange("c p q -> c (p q)"),
                          cc_out[:].rearrange("c p q -> c (p q)"))

        # ---- merge + loss ----
        # (both halves' Exp before both Ln: avoid ACT table-set swaps)
        fin_ps = psp2.tile([1, 1], f32)
        mrg, mgs, sadjs, exp_insts = [], [], [], []
        for h in range(NH):
            merged = smallp.tile([128, 4, NCORES], f32, tag=f"merged{h}")
            for st in range(4):
                pst8 = pspt.tile([128, 512], f32, tag="tps")
                nc.tensor.transpose(
                    pst8[:, :NCORES],
                    raw8[:, :, st * NH + h],
                    ident[:NCORES, :NCORES])
                nc.vector.tensor_copy(merged[:, st, :], pst8[:, :NCORES])
            mrg.append(merged)
            mg = smallp.tile([128, 1], f32, tag=f"mg{h}")
            nc.vector.tensor_reduce(mg[:], merged[:, 0, :], AX.X, ALU.max)
            mgs.append(mg)
            nmg = smallp.tile([128, 1], f32, tag=f"nmg{h}")
            nc.vector.tensor_scalar_mul(nmg[:], mg[:], -1.0)
            adj = smallp.tile([128, NCORES], f32, tag=f"adj{h}")
            exp_insts.append(
                nc.scalar.activation(adj[:], merged[:, 0, :], AF.Exp, bias=nmg[:]))
            j8 = smallp.tile([128, NCORES], f32, tag=f"j8{h}")
            nc.vector.tensor_tensor(j8[:], adj[:], merged[:, 1, :], ALU.mult)
            sadj = smallp.tile([128, 1], f32, tag=f"sadj{h}")
            nc.vector.tensor_reduce(sadj[:], j8[:], AX.X, ALU.add)
            sadjs.append(sadj)
        for h in range(NH):
            merged, mg, sadj = mrg[h], mgs[h], sadjs[h]
            lg = smallp.tile([128, 1], f32, tag=f"lg{h}")
            lg_inst = nc.scalar.activation(lg[:], sadj[:], AF.Ln)
            # keep both Exp ops before any Ln: one ACT table-set swap, not 3
            tile.add_dep_helper(lg_inst.ins, exp_insts[-1].ins, sync=False,
                                reason="group ACT table sets")
            lse = smallp.tile([128, 1], f32, tag=f"lse{h}")
            nc.vector.tensor_tensor(lse[:], lg[:], mg[:], ALU.add)
            tg = smallp.tile([128, 1], f32, tag=f"tg{h}")
            nc.vector.tensor_reduce(tg[:], merged[:, 3, :], AX.X, ALU.add)
            sg = smallp.tile([128, 1], f32, tag=f"sg{h}")
            nc.vector.tensor_reduce(sg[:], merged[:, 2, :], AX.X, ALU.add)
            a1 = smallp.tile([128, 1], f32, tag=f"a1{h}")
            nc.vector.tensor_scalar(a1[:], tg[:], -(1.0 - EPS), None, ALU.mult)
            a2 = smallp.tile([128, 1], f32, tag=f"a2{h}")
            nc.vector.tensor_scalar(a2[:], sg[:], -EPS / (P + N), None, ALU.mult)
            a3 = smallp.tile([128, 1], f32, tag=f"a3{h}")
            nc.vector.tensor_tensor(a3[:], lse[:], a1[:], ALU.add)
            lossv = smallp.tile([128, 1], f32, tag=f"loss{h}")
            nc.vector.tensor_tensor(lossv[:], a3[:], a2[:], ALU.add)
            nc.tensor.matmul(fin_ps[:], lossv[:], ones[:],
                             start=(h == 0), stop=(h == NH - 1))

        out_sb = smallp.tile([1, 1], f32, tag="outsb")
        nc.scalar.activation(out_sb[:], fin_ps[:], AF.Copy, scale=1.0 / B)
        nc.sync.dma_start(out_ext[:], out_sb[:])

    with tile.TileContext(nc) as tc:
        with (
            tc.tile_pool(name="const", bufs=1) as constp,
            tc.tile_pool(name="xp", bufs=1) as xp,
            tc.tile_pool(name="ft", bufs=3) as ftp,
            tc.tile_pool(name="stats", bufs=1) as statp,
            tc.tile_pool(name="xnp", bufs=2) as xnp,
            tc.tile_pool(name="junk", bufs=2) as scr,
            tc.tile_pool(name="mkp", bufs=2) as mkp,
            tc.tile_pool(name="small", bufs=1) as smallp,
            tc.tile_pool(name="psum", bufs=4, space="PSUM") as psp,
            tc.tile_pool(name="psumt", bufs=2, space="PSUM") as pspt,
            tc.tile_pool(name="psum2", bufs=1, space="PSUM") as psp2,
        ):
            emit(tc, constp, xp, ftp, statp, xnp, scr, mkp, smallp,
                 psp, pspt, psp2)

    nc.compile()
    return nc


def _get_compiled():
    global _COMPILED
    if _COMPILED is None:
        _COMPILED = _build()
    return _COMPILED


def kernel(inputs, targets, prototype, features):
    global LAST_RESULTS
    from concourse.bass_utils import run_bass_kernel_spmd

    inputs = np.ascontiguousarray(np.asarray(inputs, dtype=np.float32))
    prototype = np.ascontiguousarray(np.asarray(prototype, dtype=np.float32))
    features = np.asarray(features, dtype=np.float32)
    tgt = np.asarray(targets).astype(np.int64)

    iota = np.broadcast_to(np.arange(SN, dtype=np.float32), (128, SN)).copy()

    in_maps = []
    for c in range(NCORES):
        # [s, p, kc, f] tiling of features[shard].T (see kernel builder)
        featT = np.ascontiguousarray(
            features[c * NSH:(c + 1) * NSH, :].T
            .reshape(KC, 128, NSLICES, SN).transpose(2, 1, 0, 3))
        # tgtadj[b, t] = local column index target would have in slice t
        tl = tgt - c * NSH
        tgtadj = (tl[:, None] - SN * np.arange(NSLICES)[None, :]).astype(np.float32)
        in_maps.append({
            "x": inputs,
            "featT": featT,
            "proto": np.ascontiguousarray(prototype[:, c * PSH:(c + 1) * PSH]),
            "tgtadj": np.ascontiguousarray(tgtadj),
            "iota": iota,
        })

    nc = _get_compiled()
    res = run_bass_kernel_spmd(
        nc, in_maps, core_ids=list(range(NCORES)),
        trace=bool(os.environ.get("BASS_TRACE")),
    )
    LAST_RESULTS = res
    if _MERGE == "host":
        # gather per-core softmax stats [128, (st,h)] and merge
        st = np.stack([res.results[c]["out"] for c in range(NCORES)])  # [8,128,8]
        st = st.reshape(NCORES, 128, 4, NH).transpose(0, 2, 3, 1)      # [c,st,h,p]
        m, s, sm, t = (st[:, i].reshape(NCORES, B) for i in range(4))  # [c, b]
        mg = m.max(0)
        lse = mg + np.log((s * np.exp(m - mg)).sum(0))
        loss = (lse - (1 - EPS) * t.sum(0) - (EPS / (P + N)) * sm.sum(0)).mean()
        return np.float32(loss)
    return np.float32(res.results[0]["out"].reshape(()))
